# revision 4
# baseline (speedup 1.0000x reference)
"""Trainium2 Bass kernel for BaseSegHead (dynamic 1x1-conv seg logits).

Computes, for full inputs:
    qry_feats = in_feats @ qry_w.T + qry_b                  [1200, 32]
    key_map   = einsum('oc,bchw->bohw', key_w, feat_map) + key_b
    logits    = einsum('bnc,bchw->bnhw', qry_feats.reshape(4,300,32), key_map)
    out       = logits.reshape(1200, 160, 160)

Sharding: 8 cores = 4 batch images x 2 spatial (H) halves. Core c handles
batch b = c//2, rows h*80:(h+1)*80. Each core reads feat_map[b,:,rows,:]
(13.1MB), its 300 queries, and writes a [300, 80*160] output shard
(15.36MB) -- no cross-core communication and no duplicated feat_map reads.
"""

import os
import sys

sys.path.insert(0, "/opt/trn_rl_repo")
os.environ.setdefault("MYCRO_LOCAL_CACHE", "1")

import numpy as np

BATCH = 4
N_PER = 300
IN_DIM = 256
KEY_DIM = 32
FH = FW = 160
HHALF = FH // 2            # 80 rows per core
HW = HHALF * FW            # 12800 spatial positions per core
N_CORES = 8

FT = 2560                  # free-dim size of big SBUF tiles (feat / out staging)
NFT = HW // FT             # 5 big tiles per 128-channel chunk
MMN = 512                  # matmul moving free size (one PSUM bank of fp32)
PER_FT = FT // MMN         # 5 matmuls per big tile
N_CHUNKS = ((0, 128), (128, 128), (256, 44))   # query-row chunks (300 rows)

_CACHE = {}


def build_nc():
    import concourse.bass as bass
    import concourse.bacc as bacc
    import concourse.mybir as mybir
    from concourse import tile

    f32 = mybir.dt.float32
    Copy = mybir.ActivationFunctionType.Identity

    nc = bacc.Bacc("TRN2", target_bir_lowering=False, debug=False)

    featT = nc.dram_tensor("featT", [IN_DIM, HW], f32, kind="ExternalInput")
    in_featsT = nc.dram_tensor("in_featsT", [IN_DIM, N_PER], f32, kind="ExternalInput")
    qry_wT = nc.dram_tensor("qry_wT", [IN_DIM, KEY_DIM], f32, kind="ExternalInput")
    key_wT = nc.dram_tensor("key_wT", [IN_DIM, KEY_DIM], f32, kind="ExternalInput")
    qry_b = nc.dram_tensor("qry_b", [KEY_DIM, 1], f32, kind="ExternalInput")
    key_b = nc.dram_tensor("key_b", [KEY_DIM, 1], f32, kind="ExternalInput")
    out = nc.dram_tensor("out", [N_PER, HW], f32, kind="ExternalOutput")

    with tile.TileContext(nc) as tc:
        with (
            tc.tile_pool(name="const", bufs=1) as cpool,
            tc.tile_pool(name="big", bufs=10) as big,
            tc.tile_pool(name="kmap", bufs=1) as kpool,
            tc.tile_pool(name="ps_small", bufs=1, space=bass.MemorySpace.PSUM) as ps_small,
            tc.tile_pool(name="ps_main", bufs=4, space=bass.MemorySpace.PSUM) as ps_main,
        ):
            # --- small constant loads -------------------------------------
            qw0 = cpool.tile([128, KEY_DIM], f32, name="qw0")
            qw1 = cpool.tile([128, KEY_DIM], f32, name="qw1")
            kw0 = cpool.tile([128, KEY_DIM], f32, name="kw0")
            kw1 = cpool.tile([128, KEY_DIM], f32, name="kw1")
            qb = cpool.tile([KEY_DIM, 1], f32, name="qb")
            kb = cpool.tile([KEY_DIM, 1], f32, name="kb")
            inT0 = cpool.tile([128, N_PER], f32, name="inT0")
            inT1 = cpool.tile([128, N_PER], f32, name="inT1")
            nc.sync.dma_start(qw0[:], qry_wT[0:128, :])
            nc.sync.dma_start(qw1[:], qry_wT[128:256, :])
            nc.sync.dma_start(kw0[:], key_wT[0:128, :])
            nc.sync.dma_start(kw1[:], key_wT[128:256, :])
            nc.sync.dma_start(qb[:], qry_b[:])
            nc.sync.dma_start(kb[:], key_b[:])
            nc.sync.dma_start(inT0[:], in_featsT[0:128, :])
            nc.sync.dma_start(inT1[:], in_featsT[128:256, :])

            # --- qry projection: qT[c, n] = qry_w @ in_feats^T + qry_b ----
            qp = ps_small.tile([KEY_DIM, N_PER], f32, name="qp", tag="qp", bufs=1)
            nc.tensor.matmul(qp[:], qw0[:], inT0[:], start=True, stop=False)
            nc.tensor.matmul(qp[:], qw1[:], inT1[:], start=False, stop=True)
            q_sb = cpool.tile([KEY_DIM, N_PER], f32, name="q_sb")
            nc.scalar.activation(q_sb[:], qp[:], Copy, bias=qb[:])

            # --- feat_map shard loads (two 128-channel chunks x 5 tiles) --
            F = [[None] * NFT for _ in range(2)]
            for d in range(2):
                for i in range(NFT):
                    ft = big.tile([128, FT], f32, name=f"feat_{d}_{i}", tag="big")
                    nc.sync.dma_start(
                        ft[:], featT[d * 128:(d + 1) * 128, i * FT:(i + 1) * FT]
                    )
                    F[d][i] = ft

            # --- key_map[c, hw] = key_w @ feat + key_b --------------------
            key_map = kpool.tile([KEY_DIM, HW], f32, name="key_map")
            n_t = HW // MMN
            for t in range(n_t):
                kp = ps_small.tile([KEY_DIM, MMN], f32, name=f"kp_{t}", tag="kp", bufs=2)
                i, o = t // PER_FT, (t % PER_FT) * MMN
                nc.tensor.matmul(kp[:], kw0[:], F[0][i][:, o:o + MMN], start=True, stop=False)
                nc.tensor.matmul(kp[:], kw1[:], F[1][i][:, o:o + MMN], start=False, stop=True)
                nc.scalar.activation(
                    key_map[:, t * MMN:(t + 1) * MMN], kp[:], Copy, bias=kb[:]
                )

            # --- main einsum: out[n, hw] = qT.T @ key_map -----------------
            cp = 0
            for (n0, m) in N_CHUNKS:
                for i in range(NFT):
                    ot = big.tile([128, FT], f32, name=f"ot_{n0}_{i}", tag="big")
                    for j in range(PER_FT):
                        t = i * PER_FT + j
                        mp = ps_main.tile([128, MMN], f32, name=f"mp_{n0}_{t}", tag="mp")
                        nc.tensor.matmul(
                            mp[:m, :],
                            q_sb[:, n0:n0 + m],
                            key_map[:, t * MMN:(t + 1) * MMN],
                        )
                        if cp % 2 == 0:
                            nc.scalar.copy(ot[:m, j * MMN:(j + 1) * MMN], mp[:m, :])
                        else:
                            nc.vector.tensor_copy(ot[:m, j * MMN:(j + 1) * MMN], mp[:m, :])
                        cp += 1
                    nc.sync.dma_start(out[n0:n0 + m, i * FT:(i + 1) * FT], ot[:m, :])

    nc.compile()
    return nc


def _get_nc():
    if "nc" not in _CACHE:
        _CACHE["nc"] = build_nc()
    return _CACHE["nc"]


def make_in_maps(in_feats, feat_map, qry_w, qry_b, key_b, key_w):
    qry_wT = np.ascontiguousarray(qry_w.T)
    key_wT = np.ascontiguousarray(key_w.T)
    qb = np.ascontiguousarray(qry_b.reshape(KEY_DIM, 1))
    kb = np.ascontiguousarray(key_b.reshape(KEY_DIM, 1))
    in_maps = []
    for c in range(N_CORES):
        b, h = divmod(c, 2)
        in_maps.append({
            "featT": np.ascontiguousarray(
                feat_map[b, :, h * HHALF:(h + 1) * HHALF, :]
            ).reshape(IN_DIM, HW),
            "in_featsT": np.ascontiguousarray(in_feats[b * N_PER:(b + 1) * N_PER].T),
            "qry_wT": qry_wT,
            "key_wT": key_wT,
            "qry_b": qb,
            "key_b": kb,
        })
    return in_maps


def kernel(**inputs):
    in_feats = np.asarray(inputs["in_feats"], dtype=np.float32)
    feat_map = np.asarray(inputs["feat_map"], dtype=np.float32)
    qry_w = np.asarray(inputs["qry_w"], dtype=np.float32)
    qry_b = np.asarray(inputs["qry_b"], dtype=np.float32)
    key_w = np.asarray(inputs["key_w"], dtype=np.float32)
    key_b = np.asarray(inputs["key_b"], dtype=np.float32)

    from concourse import bass_utils

    nc = _get_nc()
    in_maps = make_in_maps(in_feats, feat_map, qry_w, qry_b, key_b, key_w)
    trace = os.environ.get("SEG_KERNEL_TRACE", "0") == "1"
    res = bass_utils.run_bass_kernel_spmd(
        nc, in_maps, core_ids=list(range(N_CORES)), trace=trace
    )
    _CACHE["last_result"] = res

    out = np.empty((BATCH * N_PER, FH, FW), dtype=np.float32)
    for c in range(N_CORES):
        b, h = divmod(c, 2)
        out[b * N_PER:(b + 1) * N_PER, h * HHALF:(h + 1) * HHALF, :] = (
            res.results[c]["out"].reshape(N_PER, HHALF, FW)
        )
    return out


# revision 6
# speedup vs baseline: 2.5088x; 2.5088x over previous
"""Trainium2 Bass kernel for BaseSegHead (dynamic 1x1-conv seg logits).

Computes, for full inputs:
    qry_feats = in_feats @ qry_w.T + qry_b                  [1200, 32]
    key_map   = einsum('oc,bchw->bohw', key_w, feat_map) + key_b
    logits    = einsum('bnc,bchw->bnhw', qry_feats.reshape(4,300,32), key_map)
    out       = logits.reshape(1200, 160, 160)

Sharding: 8 cores = 4 batch images x 2 spatial (H) halves. Core c handles
batch b = c//2, rows h*80:(h+1)*80. Each core reads feat_map[b,:,rows,:],
its 300 queries, and writes a [300, 80*160] output shard (15.36MB) -- no
cross-core communication and no duplicated feat_map reads.

Precision: feat_map and key_w are shipped as fp16 (the TensorEngine runs
fp32 matmuls as two half-rate passes; fp16 runs at full rate and halves
the dominant input DMA). All accumulation stays fp32 in PSUM and the
output is exact fp32 of the fp16-input product.
"""

import os
import sys

sys.path.insert(0, "/opt/trn_rl_repo")
os.environ.setdefault("MYCRO_LOCAL_CACHE", "1")

import numpy as np
import ml_dtypes

BATCH = 4
N_PER = 300
IN_DIM = 256
KEY_DIM = 32
FH = FW = 160
HHALF = FH // 2            # 80 rows per core
HW = HHALF * FW            # 12800 spatial positions per core
N_CORES = 8

FT = 2560                  # free-dim size of big SBUF tiles (feat / out staging)
NFT = HW // FT             # 5 big tiles per 128-channel chunk
MMN = 512                  # matmul moving free size (one fp32 PSUM bank)
PER_FT = FT // MMN         # 5 matmuls per big tile
N_CHUNKS = ((0, 128), (128, 128), (256, 44))   # query-row chunks (300 rows)

_CACHE = {}


def build_nc():
    import concourse.bass as bass
    import concourse.bacc as bacc
    import concourse.mybir as mybir
    from concourse import tile

    f32 = mybir.dt.float32
    bf16 = mybir.dt.float16
    Ident = mybir.ActivationFunctionType.Identity

    nc = bacc.Bacc("TRN2", target_bir_lowering=False, debug=False)

    featT = nc.dram_tensor("featT", [IN_DIM, HW], bf16, kind="ExternalInput")
    in_featsT = nc.dram_tensor("in_featsT", [IN_DIM, N_PER], f32, kind="ExternalInput")
    qry_wT = nc.dram_tensor("qry_wT", [IN_DIM, KEY_DIM], f32, kind="ExternalInput")
    key_wT = nc.dram_tensor("key_wT", [IN_DIM, KEY_DIM], bf16, kind="ExternalInput")
    qry_b = nc.dram_tensor("qry_b", [KEY_DIM, 1], f32, kind="ExternalInput")
    key_b = nc.dram_tensor("key_b", [KEY_DIM, 1], f32, kind="ExternalInput")
    out = nc.dram_tensor("out", [N_PER, HW], f32, kind="ExternalOutput")

    with tile.TileContext(nc) as tc:
        with (
            tc.tile_pool(name="const", bufs=1) as cpool,
            tc.tile_pool(name="fpool", bufs=2 * NFT) as fpool,
            tc.tile_pool(name="opool", bufs=4) as opool,
            tc.tile_pool(name="kmap", bufs=1) as kpool,
            tc.tile_pool(name="ps_small", bufs=1, space=bass.MemorySpace.PSUM) as ps_small,
            tc.tile_pool(name="ps_main", bufs=4, space=bass.MemorySpace.PSUM) as ps_main,
        ):
            # --- small constant loads -------------------------------------
            qw0 = cpool.tile([128, KEY_DIM], f32, name="qw0")
            qw1 = cpool.tile([128, KEY_DIM], f32, name="qw1")
            kw0 = cpool.tile([128, KEY_DIM], bf16, name="kw0")
            kw1 = cpool.tile([128, KEY_DIM], bf16, name="kw1")
            qb = cpool.tile([KEY_DIM, 1], f32, name="qb")
            kb = cpool.tile([KEY_DIM, 1], f32, name="kb")
            inT0 = cpool.tile([128, N_PER], f32, name="inT0")
            inT1 = cpool.tile([128, N_PER], f32, name="inT1")
            nc.sync.dma_start(qw0[:], qry_wT[0:128, :])
            nc.sync.dma_start(qw1[:], qry_wT[128:256, :])
            nc.sync.dma_start(kw0[:], key_wT[0:128, :])
            nc.sync.dma_start(kw1[:], key_wT[128:256, :])
            nc.sync.dma_start(qb[:], qry_b[:])
            nc.sync.dma_start(kb[:], key_b[:])
            nc.sync.dma_start(inT0[:], in_featsT[0:128, :])
            nc.sync.dma_start(inT1[:], in_featsT[128:256, :])

            # --- qry projection: qT[c, n] = qry_w @ in_feats^T + qry_b ----
            qp = ps_small.tile([KEY_DIM, N_PER], f32, name="qp", tag="qp", bufs=1)
            nc.tensor.matmul(qp[:], qw0[:], inT0[:], start=True, stop=False)
            nc.tensor.matmul(qp[:], qw1[:], inT1[:], start=False, stop=True)
            q_sb = cpool.tile([KEY_DIM, N_PER], bf16, name="q_sb")
            nc.scalar.activation(q_sb[:], qp[:], Ident, bias=qb[:])

            # --- feat_map shard loads, (d0,d1) pairs interleaved ----------
            F = [[None] * NFT for _ in range(2)]
            for i in range(NFT):
                for d in range(2):
                    ft = fpool.tile([128, FT], bf16, name=f"feat_{d}_{i}", tag="fbf")
                    nc.sync.dma_start(
                        ft[:], featT[d * 128:(d + 1) * 128, i * FT:(i + 1) * FT]
                    )
                    F[d][i] = ft

            # --- key_map[c, hw] = key_w @ feat + key_b  (bf16 out) --------
            key_map = kpool.tile([KEY_DIM, HW], bf16, name="key_map")
            n_t = HW // MMN
            for t in range(n_t):
                kp = ps_small.tile([KEY_DIM, MMN], f32, name=f"kp_{t}", tag="kp", bufs=2)
                i, o = t // PER_FT, (t % PER_FT) * MMN
                nc.tensor.matmul(kp[:], kw0[:], F[0][i][:, o:o + MMN], start=True, stop=False)
                nc.tensor.matmul(kp[:], kw1[:], F[1][i][:, o:o + MMN], start=False, stop=True)
                nc.scalar.activation(
                    key_map[:, t * MMN:(t + 1) * MMN], kp[:], Ident, bias=kb[:]
                )

            # --- main einsum: out[n, hw] = qT.T @ key_map -----------------
            cp = 0
            for (n0, m) in N_CHUNKS:
                for i in range(NFT):
                    ot = opool.tile([128, FT], f32, name=f"ot_{n0}_{i}", tag="obuf")
                    for j in range(PER_FT):
                        t = i * PER_FT + j
                        mp = ps_main.tile([128, MMN], f32, name=f"mp_{n0}_{t}", tag="mp")
                        nc.tensor.matmul(
                            mp[:m, :],
                            q_sb[:, n0:n0 + m],
                            key_map[:, t * MMN:(t + 1) * MMN],
                        )
                        if cp % 2 == 0:
                            nc.scalar.copy(ot[:m, j * MMN:(j + 1) * MMN], mp[:m, :])
                        else:
                            nc.vector.tensor_copy(ot[:m, j * MMN:(j + 1) * MMN], mp[:m, :])
                        cp += 1
                    nc.gpsimd.dma_start(out[n0:n0 + m, i * FT:(i + 1) * FT], ot[:m, :])

    nc.compile()
    return nc


def _get_nc():
    if "nc" not in _CACHE:
        _CACHE["nc"] = build_nc()
    return _CACHE["nc"]


def make_in_maps(in_feats, feat_map, qry_w, qry_b, key_b, key_w):
    qry_wT = np.ascontiguousarray(qry_w.T)
    key_wT = np.ascontiguousarray(key_w.T).astype(np.float16)
    qb = np.ascontiguousarray(qry_b.reshape(KEY_DIM, 1))
    kb = np.ascontiguousarray(key_b.reshape(KEY_DIM, 1))
    in_maps = []
    for c in range(N_CORES):
        b, h = divmod(c, 2)
        in_maps.append({
            "featT": np.ascontiguousarray(
                feat_map[b, :, h * HHALF:(h + 1) * HHALF, :]
            ).reshape(IN_DIM, HW).astype(np.float16),
            "in_featsT": np.ascontiguousarray(in_feats[b * N_PER:(b + 1) * N_PER].T),
            "qry_wT": qry_wT,
            "key_wT": key_wT,
            "qry_b": qb,
            "key_b": kb,
        })
    return in_maps


def kernel(**inputs):
    in_feats = np.asarray(inputs["in_feats"], dtype=np.float32)
    feat_map = np.asarray(inputs["feat_map"], dtype=np.float32)
    qry_w = np.asarray(inputs["qry_w"], dtype=np.float32)
    qry_b = np.asarray(inputs["qry_b"], dtype=np.float32)
    key_w = np.asarray(inputs["key_w"], dtype=np.float32)
    key_b = np.asarray(inputs["key_b"], dtype=np.float32)

    from concourse import bass_utils

    nc = _get_nc()
    in_maps = make_in_maps(in_feats, feat_map, qry_w, qry_b, key_b, key_w)
    trace = os.environ.get("SEG_KERNEL_TRACE", "0") == "1"
    res = bass_utils.run_bass_kernel_spmd(
        nc, in_maps, core_ids=list(range(N_CORES)), trace=trace
    )
    _CACHE["last_result"] = res

    out = np.empty((BATCH * N_PER, FH, FW), dtype=np.float32)
    for c in range(N_CORES):
        b, h = divmod(c, 2)
        out[b * N_PER:(b + 1) * N_PER, h * HHALF:(h + 1) * HHALF, :] = (
            res.results[c]["out"].reshape(N_PER, HHALF, FW)
        )
    return out


# revision 7
# speedup vs baseline: 2.9584x; 1.1792x over previous
"""Trainium2 Bass kernel for BaseSegHead (dynamic 1x1-conv seg logits).

Computes, for full inputs:
    qry_feats = in_feats @ qry_w.T + qry_b                  [1200, 32]
    key_map   = einsum('oc,bchw->bohw', key_w, feat_map) + key_b
    logits    = einsum('bnc,bchw->bnhw', qry_feats.reshape(4,300,32), key_map)
    out       = logits.reshape(1200, 160, 160)

Sharding: 8 cores = 4 batch images x 2 spatial (H) halves. Core c handles
batch b = c//2, rows h*80:(h+1)*80. Each core reads feat_map[b,:,rows,:],
its 300 queries, and writes a [300, 80*160] output shard (15.36MB) -- no
cross-core communication and no duplicated feat_map reads.

Precision: feat_map and key_w are shipped as fp16 (the TensorEngine runs
fp32 matmuls as two half-rate passes; fp16 runs at full rate and halves
the dominant input DMA). All accumulation stays fp32 in PSUM and the
output is exact fp32 of the fp16-input products.

Small constants (projection weights, biases, transposed queries) are
host-packed into two tensors so startup is 2 DMAs instead of 8, and the
first feat pair is enqueued right behind them so the key_map matmul
stream starts as early as possible.
"""

import os
import sys

sys.path.insert(0, "/opt/trn_rl_repo")
os.environ.setdefault("MYCRO_LOCAL_CACHE", "1")

import numpy as np

BATCH = 4
N_PER = 300
IN_DIM = 256
KEY_DIM = 32
FH = FW = 160
HHALF = FH // 2            # 80 rows per core
HW = HHALF * FW            # 12800 spatial positions per core
N_CORES = 8

FT = 2560                  # free-dim size of big SBUF tiles (feat / out staging)
NFT = HW // FT             # 5 big tiles per 128-channel chunk
MMN = 512                  # matmul moving free size (one fp32 PSUM bank)
PER_FT = FT // MMN         # 5 matmuls per big tile
N_CHUNKS = ((0, 128), (128, 128), (256, 44))   # query-row chunks (300 rows)
CPACK_W = 666              # qry_wT (64) + in_featsT (600) + qry_b (1) + key_b (1)

_CACHE = {}


def build_nc():
    import concourse.bass as bass
    import concourse.bacc as bacc
    import concourse.mybir as mybir
    from concourse import tile

    f32 = mybir.dt.float32
    f16 = mybir.dt.float16
    Ident = mybir.ActivationFunctionType.Identity

    nc = bacc.Bacc("TRN2", target_bir_lowering=False, debug=False)

    featT = nc.dram_tensor("featT", [IN_DIM, HW], f16, kind="ExternalInput")
    cpack = nc.dram_tensor("cpack", [128, CPACK_W], f32, kind="ExternalInput")
    kwp = nc.dram_tensor("kwp", [128, 2 * KEY_DIM], f16, kind="ExternalInput")
    out = nc.dram_tensor("out", [N_PER, HW], f32, kind="ExternalOutput")

    with tile.TileContext(nc) as tc:
        with (
            tc.tile_pool(name="const", bufs=1) as cpool,
            tc.tile_pool(name="fpool", bufs=2 * NFT) as fpool,
            tc.tile_pool(name="opool", bufs=4) as opool,
            tc.tile_pool(name="kmap", bufs=1) as kpool,
            tc.tile_pool(name="ps_small", bufs=2, space=bass.MemorySpace.PSUM) as ps_small,
            tc.tile_pool(name="ps_main", bufs=6, space=bass.MemorySpace.PSUM) as ps_main,
        ):
            # --- loads: key weights + first feat pair first ---------------
            kwt = cpool.tile([128, 2 * KEY_DIM], f16, name="kwt")
            nc.sync.dma_start(kwt[:], kwp[:])
            kw = (kwt[:, 0:KEY_DIM], kwt[:, KEY_DIM:2 * KEY_DIM])

            F = [[None] * NFT for _ in range(2)]

            def load_pair(i):
                for d in range(2):
                    ft = fpool.tile([128, FT], f16, name=f"feat_{d}_{i}", tag="fbf")
                    nc.sync.dma_start(
                        ft[:], featT[d * 128:(d + 1) * 128, i * FT:(i + 1) * FT]
                    )
                    F[d][i] = ft

            load_pair(0)

            ct = cpool.tile([128, CPACK_W], f32, name="ct")
            nc.sync.dma_start(ct[:], cpack[:])
            qw = (ct[:, 0:32], ct[:, 32:64])
            inT = (ct[:, 64:364], ct[:, 364:664])
            qb = ct[0:KEY_DIM, 664:665]
            kb = ct[0:KEY_DIM, 665:666]

            for i in range(1, NFT):
                load_pair(i)

            # --- qry projection: qT[c, n] = qry_w @ in_feats^T + qry_b ----
            qp = ps_small.tile([KEY_DIM, MMN], f32, name="qp", tag="kp")
            nc.tensor.matmul(qp[:, 0:N_PER], qw[0], inT[0], start=True, stop=False)
            nc.tensor.matmul(qp[:, 0:N_PER], qw[1], inT[1], start=False, stop=True)
            q_sb = cpool.tile([KEY_DIM, N_PER], f16, name="q_sb")
            nc.scalar.activation(q_sb[:], qp[:, 0:N_PER], Ident, bias=qb)

            # --- key_map[c, hw] = key_w @ feat + key_b  (fp16 out) --------
            key_map = kpool.tile([KEY_DIM, HW], f16, name="key_map")
            n_t = HW // MMN
            for t in range(n_t):
                kp = ps_small.tile([KEY_DIM, MMN], f32, name=f"kp_{t}", tag="kp")
                i, o = t // PER_FT, (t % PER_FT) * MMN
                nc.tensor.matmul(kp[:], kw[0], F[0][i][:, o:o + MMN], start=True, stop=False)
                nc.tensor.matmul(kp[:], kw[1], F[1][i][:, o:o + MMN], start=False, stop=True)
                nc.scalar.activation(
                    key_map[:, t * MMN:(t + 1) * MMN], kp[:], Ident, bias=kb
                )

            # --- main einsum: out[n, hw] = qT.T @ key_map -----------------
            cp = 0
            for (n0, m) in N_CHUNKS:
                for i in range(NFT):
                    ot = opool.tile([128, FT], f32, name=f"ot_{n0}_{i}", tag="obuf")
                    for j in range(PER_FT):
                        t = i * PER_FT + j
                        mp = ps_main.tile([128, MMN], f32, name=f"mp_{n0}_{t}", tag="mp")
                        nc.tensor.matmul(
                            mp[:m, :],
                            q_sb[:, n0:n0 + m],
                            key_map[:, t * MMN:(t + 1) * MMN],
                        )
                        if cp % 3 == 0:
                            nc.scalar.copy(ot[:m, j * MMN:(j + 1) * MMN], mp[:m, :])
                        else:
                            nc.vector.tensor_copy(ot[:m, j * MMN:(j + 1) * MMN], mp[:m, :])
                        cp += 1
                    nc.sync.dma_start(out[n0:n0 + m, i * FT:(i + 1) * FT], ot[:m, :])

    nc.compile()
    return nc


def _get_nc():
    if "nc" not in _CACHE:
        _CACHE["nc"] = build_nc()
    return _CACHE["nc"]


def make_in_maps(in_feats, feat_map, qry_w, qry_b, key_b, key_w):
    kwT = np.ascontiguousarray(key_w.T).astype(np.float16)   # [256, 32]
    kwp = np.concatenate([kwT[0:128], kwT[128:256]], axis=1)  # [128, 64]
    kwp = np.ascontiguousarray(kwp)
    qwT = qry_w.T.astype(np.float32)                          # [256, 32]
    in_maps = []
    for c in range(N_CORES):
        b, h = divmod(c, 2)
        ifT = in_feats[b * N_PER:(b + 1) * N_PER].T           # [256, 300]
        cpack = np.zeros((128, CPACK_W), np.float32)
        cpack[:, 0:32] = qwT[0:128]
        cpack[:, 32:64] = qwT[128:256]
        cpack[:, 64:364] = ifT[0:128]
        cpack[:, 364:664] = ifT[128:256]
        cpack[0:KEY_DIM, 664] = qry_b
        cpack[0:KEY_DIM, 665] = key_b
        in_maps.append({
            "featT": np.ascontiguousarray(
                feat_map[b, :, h * HHALF:(h + 1) * HHALF, :]
            ).reshape(IN_DIM, HW).astype(np.float16),
            "cpack": cpack,
            "kwp": kwp,
        })
    return in_maps


def kernel(**inputs):
    in_feats = np.asarray(inputs["in_feats"], dtype=np.float32)
    feat_map = np.asarray(inputs["feat_map"], dtype=np.float32)
    qry_w = np.asarray(inputs["qry_w"], dtype=np.float32)
    qry_b = np.asarray(inputs["qry_b"], dtype=np.float32)
    key_w = np.asarray(inputs["key_w"], dtype=np.float32)
    key_b = np.asarray(inputs["key_b"], dtype=np.float32)

    from concourse import bass_utils

    nc = _get_nc()
    in_maps = make_in_maps(in_feats, feat_map, qry_w, qry_b, key_b, key_w)
    trace = os.environ.get("SEG_KERNEL_TRACE", "0") == "1"
    res = bass_utils.run_bass_kernel_spmd(
        nc, in_maps, core_ids=list(range(N_CORES)), trace=trace
    )
    _CACHE["last_result"] = res

    out = np.empty((BATCH * N_PER, FH, FW), dtype=np.float32)
    for c in range(N_CORES):
        b, h = divmod(c, 2)
        out[b * N_PER:(b + 1) * N_PER, h * HHALF:(h + 1) * HHALF, :] = (
            res.results[c]["out"].reshape(N_PER, HHALF, FW)
        )
    return out
